# revision 11
# baseline (speedup 1.0000x reference)
"""FALCON ObjectSomeValuesFrom forward kernel for Trainium2 (Bass/Tile).

Math: the reference computes
    c_fs[j]   = sigmoid(cw + col_j + b)
    r_fs[i,j] = sigmoid(row_i + col_j + b)
    out[i]    = max_j r_fs[i,j] * c_fs[j]
with col_j = e_j . w_r, row_i = e_i . w_l + rw, cw = c_emb . w_l,
rw = r_emb . w_l.  Both product factors are strictly increasing in col_j,
so the max over j is attained at argmax_j col_j for every i:
    out[i] = sigmoid(a_i + rw + colmax + b) * sigmoid(cw + colmax + b)
with a_i = e_i . w_l and colmax = max_j col_j.  The O(N^2) pairwise block
collapses to two GEMVs over e_all plus an elementwise sigmoid tail.

Sharding: the entity axis is split 8 ways (1024 rows per core).  Each
core loads ONLY its own shard — [128, 2+1024] fp8-e3m4 (w_r, w_l in the
first two columns, the transposed shard after) — computes both GEMVs as
8 PE matmuls ([K=128, M=128] stationary x [K=128, N=2] moving, PSUM
[128, 8, 2]), and a DVE free-axis reduce_max over its 8 col-dot columns
(the shard-local max-reduction over j).  The raw PSUM block [128, 17]
(8x2 dots + the 128 partial maxima) is DMA'd out in f32.  The host-side
gather/unshard combines the 8 shards: colmax = max over the 8x128
partial maxima (8 scalars of real reduction work), then the elementwise
sigmoid finish over the gathered a-vector.  No cross-device
communication — the cross-core max rides the output gather, per the
sharding plan.

Why sharded: every-core-scans-the-full-table (the previous design) is
bound by the serialized 1 MB HBM read — 2.9 us of DMA on top of a
~6.1 us fixed-latency floor (DMA issue 650 + HWDGE 625 + DGE 650 +
completion semaphore 900 per direction, plus pre/postamble barriers).
An 8x smaller shard read (365 ns) puts the kernel at that floor, and a
cross-core collective instead would cost ~28 us (AllReduce constant
overhead), far more than it saves.

Critical path per core: preamble -> one input DMA (SP/HWDGE) -> 8
matmul pairs -> one DVE reduce -> one output DMA.  The two GEMV dots
land interleaved in PSUM ([p, 2c] = col-dot, [p, 2c+1] = a-dot of shard
entity c*128+p) and the DVE max lands in column 16, so a single
contiguous [128, 17] f32 DMA ships everything (descriptor floor,
~56 ns) with no SBUF copy, no activation-table load, and no Pool
partition-reduce on the critical path.
"""

import numpy as np

N = 8192        # 8000 named + 192 anon entities
D = 128         # emb dim == contraction == partitions
P = 128
NCORES = 8
RPC = N // NCORES     # rows per core (1024)
OWN = RPC // P        # 8 chunks of 128 rows per core
OUTC = OWN + 1        # 8 a-dot columns + 1 max column
COL_DT = "fp8e3"      # "fp8e4" | "fp8e3" | "fp16" | "bf16"
SCALE = {"fp8e4": 8.0, "fp8e3": 32.0, "fp16": 1.0, "bf16": 1.0}

_CACHE = {}


def _np_dt(col_dt):
    import ml_dtypes
    return {
        "fp8e4": ml_dtypes.float8_e4m3,
        "fp8e3": ml_dtypes.float8_e3m4,
        "fp16": np.float16,
        "bf16": ml_dtypes.bfloat16,
    }[col_dt]


def _build_nc(repeat=1, col_dt=COL_DT):
    import concourse.bass as bass  # noqa: F401  (env preload)
    import concourse.bacc as bacc
    import concourse.tile as tile
    import concourse.mybir as mybir

    f32 = mybir.dt.float32
    cdt = {
        "fp8e4": mybir.dt.float8e4,
        "fp8e3": mybir.dt.float8e3,
        "fp16": mybir.dt.float16,
        "bf16": mybir.dt.bfloat16,
    }[col_dt]
    nc = bacc.Bacc("TRN2", target_bir_lowering=False, debug=False)

    # Cols 0:2 = [w_r, w_l] (scaled); 2:RPC+2 = this core's shard of the
    # scaled/transposed e-table.  One DMA covers every input.
    et_d = nc.dram_tensor("et", [P, RPC + 2], cdt, kind="ExternalInput").ap()
    out_d = nc.dram_tensor("out", [P, OUTC], f32, kind="ExternalOutput").ap()

    with tile.TileContext(nc) as tc:
        with (
            tc.tile_pool(name="sb", bufs=1) as sb,
            tc.tile_pool(name="ps", bufs=1, space="PSUM") as ps,
        ):
            et = sb.tile([P, RPC + 2], cdt)
            nc.sync.dma_start(et[:], et_d[:])

            # Dependency-free dummy copy: if the compiler decides the
            # Activation engine needs an act-table load for Copy, it lands
            # here, inside the DMA window, not on the critical tail.
            dum = sb.tile([P, 1], f32)
            nc.vector.memset(dum[:], 0.0)
            dum2 = sb.tile([P, 1], f32)
            nc.scalar.copy(dum2[:], dum[:])

            # Two PSUM tiles, one per GEMV: the tail's two readers (Act
            # copy of psA, DVE reduce of psB) would be falsely serialized
            # by tile's cross-engine same-PSUM-tile read hazard otherwise.
            w_r = et[:, 0:1]
            w_l = et[:, 1:2]
            psA = ps.tile([P, OWN], f32)   # a-dots  (w_l)
            psB = ps.tile([P, OWN], f32)   # col-dots (w_r)
            # psA matmuls first: the Act copy (the longer tail op) waits
            # only on these 8, releasing ~30ns earlier than interleaved.
            for _ in range(repeat):
                for c in range(OWN):
                    chunk = et[:, 2 + c * P : 2 + (c + 1) * P]
                    nc.tensor.matmul(
                        psA[:, c : c + 1], chunk, w_l, start=True, stop=True
                    )
                for c in range(OWN):
                    chunk = et[:, 2 + c * P : 2 + (c + 1) * P]
                    nc.tensor.matmul(
                        psB[:, c : c + 1], chunk, w_r, start=True, stop=True
                    )

            # PSUM cannot DMA to DRAM directly: stage through SBUF.  The
            # a-dots copy (Activation engine) runs in parallel with the
            # shard-local max over j (DVE, per-partition max of the 8
            # col-dot columns); they land adjacently so one DMA ships both.
            ot = sb.tile([P, OUTC], f32)
            nc.scalar.copy(ot[:, 0:OWN], psA[:])
            nc.vector.reduce_max(
                ot[:, OWN:OUTC], psB[:], axis=mybir.AxisListType.X
            )

            nc.sync.dma_start(out_d[:], ot[:])

    nc.compile()
    return nc


def get_nc(repeat=1, col_dt=COL_DT):
    key = ("nc", repeat, col_dt)
    if key not in _CACHE:
        _CACHE[key] = _build_nc(repeat, col_dt)
    return _CACHE[key]


def prep(anon_e_emb, e_table, c_table, r_table, fc0_w, fc0_b, c_id, r_id,
         col_dt=COL_DT):
    """Host prep: shard + quantize inputs; return (in_maps, aux for finish)."""
    e_all = np.concatenate(
        [np.asarray(e_table, np.float32), np.asarray(anon_e_emb, np.float32)], 0
    )  # [N, D]
    fc0_w = np.asarray(fc0_w, np.float32)
    w_l = fc0_w[0, :D]
    w_r = fc0_w[0, D:]
    b = np.float32(np.asarray(fc0_b, np.float32)[0])
    c_emb = np.asarray(c_table, np.float32)[int(c_id)]
    r_emb = np.asarray(r_table, np.float32)[int(r_id)]
    rw = np.float32(np.dot(r_emb, w_l))
    cw = np.float32(np.dot(c_emb, w_l))

    s = SCALE[col_dt]
    ndt = _np_dt(col_dt)
    eT = np.ascontiguousarray((e_all.T * s).astype(ndt))  # [D, N] quantized
    wq = np.stack([(w_r * s).astype(ndt), (w_l * s).astype(ndt)], axis=1)

    in_maps = []
    for core in range(NCORES):
        aug = np.empty((P, RPC + 2), ndt)
        aug[:, 0:2] = wq
        aug[:, 2:] = eT[:, core * RPC : (core + 1) * RPC]
        in_maps.append({"et": np.ascontiguousarray(aug)})

    aux = {"rw": rw, "cw": cw, "b": b, "inv_s2": np.float32(1.0 / (s * s))}
    return in_maps, aux


def host_finish(core_outs, aux):
    """Gather/unshard: 8-scalar max across shards + elementwise finish.

    core_outs[c] is core c's [128, 9] f32 block: [p, c] = a-dot of
    shard entity c*128+p (x S^2), [p, 8] = shard-local col max (x S^2).
    """
    inv_s2 = aux["inv_s2"]
    colmax = max(o[:, OWN].max() for o in core_outs) * inv_s2
    a = np.concatenate(
        [o[:, 0:OWN].T.reshape(-1) for o in core_outs]
    ) * inv_s2

    def sigmoid(x):
        return 1.0 / (1.0 + np.exp(-x))

    k1 = a + (aux["rw"] + aux["b"] + colmax)
    k2 = sigmoid(np.float32(aux["cw"] + aux["b"]) + colmax)
    return (sigmoid(k1) * k2).astype(np.float32)


def run(inputs, trace=False, trace_kwargs=None, repeat=1, col_dt=COL_DT):
    from concourse.bass_utils import run_bass_kernel_spmd

    nc = get_nc(repeat, col_dt)
    in_maps, aux = prep(**inputs, col_dt=col_dt)
    res = run_bass_kernel_spmd(
        nc,
        in_maps,
        core_ids=list(range(NCORES)),
        trace=trace,
        **(trace_kwargs or {}),
    )
    out = host_finish(
        [np.asarray(res.results[c]["out"], np.float32) for c in range(NCORES)],
        aux,
    )
    return out, res


def kernel(**inputs) -> np.ndarray:
    out, _ = run(inputs, trace=False)
    return out


# revision 31
# speedup vs baseline: 1.2317x; 1.2317x over previous
"""FALCON ObjectSomeValuesFrom forward kernel for Trainium2 (Bass, raw).

Math: the reference computes
    c_fs[j]   = sigmoid(cw + col_j + b)
    r_fs[i,j] = sigmoid(row_i + col_j + b)
    out[i]    = max_j r_fs[i,j] * c_fs[j]
with col_j = e_j . w_r, row_i = e_i . w_l + rw, cw = c_emb . w_l,
rw = r_emb . w_l.  Both product factors are strictly increasing in col_j,
so the max over j is attained at argmax_j col_j for every i:
    out[i] = sigmoid(a_i + rw + colmax + b) * sigmoid(cw + colmax + b)
with a_i = e_i . w_l and colmax = max_j col_j.  The O(N^2) pairwise block
collapses to two GEMVs over e_all plus an elementwise sigmoid tail.

Sharding: the entity axis is split 8 ways (1024 rows per core).  Each
core loads ONLY its own shard — [128, 2+1024] fp8-e3m4 (w_r, w_l in the
first two columns, the transposed shard after) — computes both GEMVs as
8 N=2 PE matmuls into one interleaved PSUM tile ([p, 2c] = col-dot,
[p, 2c+1] = a-dot of shard entity c*128+p), then in parallel: the
Activation engine copies the a-dots PSUM->SBUF while DVE max-reduces
the col-dots (the shard-local max-reduction over j).  The host-side gather/unshard combines the 8 shards: colmax = max
over the 8x128 partial maxima, then the elementwise sigmoid finish over
the gathered a-vector.  No cross-device communication — the cross-core
max rides the output gather, per the sharding plan (a device collective
would cost ~28 us of AllReduce overhead, far more than it saves).

Why raw Bass (no TileContext): the kernel is pure fixed latency — one
365 ns input DMA (+650 SEQ, +650 DGE, +900 completion semaphore),
~70 ns of matmuls, ~520 ns of tail compute+sem-props, then the output
DMA chain (625 HWDGE + 650 DGE + 900 sem) — so framework overhead IS
the cost.  Hand-rolled semaphores shed the TileContext's extra
pre/mid/post barriers and sync-point instructions (~230 ns), and
suppressing the Bass constructor's const-AP preamble (4 memsets + an
all-engine barrier this kernel never needs) lets every engine start at
t~0, shifting the whole pipeline ~640 ns left.  (A prepped-SWDGE
scatter + trigger_dma output path that also skips the 1275 ns
HWDGE+DGE stages simulates at 5149 ns and is CoreSim-exact, but the
axon/NRT backend dies with NRT_EXEC_UNIT_UNRECOVERABLE executing
gen_mode=1 preps, so it cannot be shipped.)

Sem protocol (one per edge; producers signal via drain().then_inc —
an engine-empty drain's SEQ-side update propagates ~90 ns faster than
an engine-op-attached update; sems reset by an end barrier + range
clear whose cross-engine rendezvous overlaps the final DMA wait):
  SP:   dma_start(et).then_inc(in_sem, 16)
  PE:   wait in_sem; 8x N=2 matmuls; drain (+pe_sem)
  Act:  wait pe_sem; copy a-dots -> ot[:, 0:8]; drain (+act_sem)
  DVE:  wait pe_sem; reduce_max col-dots -> ot[:, 8]; drain (+dve_sem)
  SP:   wait act+dve; dma_start(out); wait dma_sem; barrier; sem clear
"""

import numpy as np

N = 8192        # 8000 named + 192 anon entities
D = 128         # emb dim == contraction == partitions
P = 128
NCORES = 8
RPC = N // NCORES     # rows per core (1024)
OWN = RPC // P        # 8 chunks of 128 rows per core
ELEM = 16             # out columns (64B/partition keeps min-time descriptors)
COL_DT = "fp8e3"      # "fp8e4" | "fp8e3" | "fp16" | "bf16"
SCALE = {"fp8e4": 8.0, "fp8e3": 32.0, "fp16": 1.0, "bf16": 1.0}

_CACHE = {}


def _np_dt(col_dt):
    import ml_dtypes
    return {
        "fp8e4": ml_dtypes.float8_e4m3,
        "fp8e3": ml_dtypes.float8_e3m4,
        "fp16": np.float16,
        "bf16": ml_dtypes.bfloat16,
    }[col_dt]


def _build_nc(repeat=1, col_dt=COL_DT):
    import concourse.bass as bass  # noqa: F401  (env preload)
    import concourse.bacc as bacc
    import concourse.mybir as mybir

    f32 = mybir.dt.float32
    cdt = {
        "fp8e4": mybir.dt.float8e4,
        "fp8e3": mybir.dt.float8e3,
        "fp16": mybir.dt.float16,
        "bf16": mybir.dt.bfloat16,
    }[col_dt]

    # The Bass constructor unconditionally emits 4 const-AP memsets plus an
    # all-engine barrier — ~640 ns of preamble before any engine may start.
    # This kernel never reads the const APs (the one zero-source it needs is
    # hand-rolled below with a single-sem handshake), so suppress that
    # emission during construction; every engine then starts at t~0 and the
    # input DMA issues ~640 ns earlier.
    _orig_memset = bass.BassGpSimd.memset
    _orig_barrier = bass.Bass.all_engine_barrier
    bass.BassGpSimd.memset = lambda self, ap, c: None
    bass.Bass.all_engine_barrier = lambda self: None
    try:
        nc = bacc.Bacc("TRN2", target_bir_lowering=False, debug=False)
    finally:
        bass.BassGpSimd.memset = _orig_memset
        bass.Bass.all_engine_barrier = _orig_barrier

    # Cols 0:2 = [w_r, w_l] (scaled); 2:RPC+2 = this core's shard of the
    # scaled/transposed e-table.  One DMA covers every input.
    et_d = nc.dram_tensor("et", [P, RPC + 2], cdt, kind="ExternalInput").ap()
    out_d = nc.dram_tensor("out", [P, ELEM], f32, kind="ExternalOutput").ap()

    et = nc.alloc_sbuf_tensor("etsb", [P, RPC + 2], cdt).ap()
    ot = nc.alloc_sbuf_tensor("ot", [P, ELEM], f32).ap()
    zsrc = nc.alloc_sbuf_tensor("zsrc", [P, 1], f32).ap()
    dum2 = nc.alloc_sbuf_tensor("dum2", [P, 1], f32).ap()
    pst = nc.alloc_psum_tensor("pst", [P, 2 * OWN], f32).ap()
    psv = pst.rearrange("p (n two) -> p n two", two=2)

    z_sem = nc.alloc_semaphore("z_sem")
    in_sem = nc.alloc_semaphore("in_sem")
    pe_sem = nc.alloc_semaphore("pe_sem")
    act_sem = nc.alloc_semaphore("act_sem")
    dve_sem = nc.alloc_semaphore("dve_sem")
    dma_sem = nc.alloc_semaphore("dma_sem")
    sems = [z_sem, in_sem, pe_sem, act_sem, dve_sem, dma_sem]

    # SP: the one input DMA
    nc.sync.dma_start(et, et_d).then_inc(in_sem, 16)

    # DVE: zero the pad columns of the staging tile (cols 9..15) and the
    # dummy-copy source
    nc.vector.memset(ot[:, OWN + 1 :], 0.0)
    nc.vector.memset(zsrc, 0.0).then_inc(z_sem, 1)

    # Act: dependency-free dummy copy hoists the act-table load (which the
    # compiler inserts before the first Activation) into the DMA window
    nc.scalar.wait_ge(z_sem, 1)
    nc.scalar.copy(dum2, zsrc)

    # PE: both GEMVs as 8 N=2 matmuls ([p, 2c]=col-dot, [p, 2c+1]=a-dot).
    # One PSUM tile: raw bass has no cross-engine PSUM read hazard (the
    # Tile framework's false serialization forced a two-tile split).
    nc.tensor.wait_ge(in_sem, 16)
    # in_sem is deliberately not consumed mid-run: the race detector
    # treats a dec of a DMA-completion sem as unsafe while the write-
    # record lives; the end-of-program range clear resets it instead.
    w2 = et[:, 0:2]
    for _ in range(repeat):
        for c in range(OWN):
            chunk = et[:, 2 + c * P : 2 + (c + 1) * P]
            nc.tensor.matmul(psv[:, c, :], chunk, w2,
                             start=True, stop=True)
    # a single PE drain signals engine-empty (all matmuls retired);
    # its SEQ-side update propagates faster than an engine-op update
    nc.tensor.drain().then_inc(pe_sem, 1)

    # Act: a-dots copy PSUM->SBUF (drain-signaled)
    nc.scalar.wait_ge(pe_sem, 1)
    nc.scalar.copy(ot[:, 0:OWN], psv[:, :, 1])
    nc.scalar.drain().then_inc(act_sem, 1)

    # DVE: shard-local max over j (drain-signaled)
    nc.vector.wait_ge(pe_sem, 1)
    nc.vector.reduce_max(ot[:, OWN : OWN + 1], psv[:, :, 0],
                         axis=mybir.AxisListType.X)
    nc.vector.drain().then_inc(dve_sem, 1)

    # SP: the output DMA, gated on both producers
    nc.sync.wait_ge(act_sem, 1)
    nc.sync.wait_ge(dve_sem, 1)
    nc.sync.dma_start(out_d, ot).then_inc(dma_sem, 16)
    nc.sync.wait_ge(dma_sem, 16)
    # reset all sems for NEFF re-execution: the full two-phase all-engine
    # barrier (the race detector requires the release round-trip before a
    # sem clear), then one range clear on Pool.  The rendezvous overlaps
    # the dma_sem wait (other engines drain early), so only the short
    # release/clear tail lands on the critical path.
    nc.all_engine_barrier()
    nc.gpsimd.sem_clear(range(sems[0].num, sems[-1].num + 1))

    nc.compile()
    return nc


def get_nc(repeat=1, col_dt=COL_DT):
    key = ("nc", repeat, col_dt)
    if key not in _CACHE:
        _CACHE[key] = _build_nc(repeat, col_dt)
    return _CACHE[key]


def prep(anon_e_emb, e_table, c_table, r_table, fc0_w, fc0_b, c_id, r_id,
         col_dt=COL_DT):
    """Host prep: shard + quantize inputs; return (in_maps, aux for finish)."""
    e_all = np.concatenate(
        [np.asarray(e_table, np.float32), np.asarray(anon_e_emb, np.float32)], 0
    )  # [N, D]
    fc0_w = np.asarray(fc0_w, np.float32)
    w_l = fc0_w[0, :D]
    w_r = fc0_w[0, D:]
    b = np.float32(np.asarray(fc0_b, np.float32)[0])
    c_emb = np.asarray(c_table, np.float32)[int(c_id)]
    r_emb = np.asarray(r_table, np.float32)[int(r_id)]
    rw = np.float32(np.dot(r_emb, w_l))
    cw = np.float32(np.dot(c_emb, w_l))

    s = SCALE[col_dt]
    ndt = _np_dt(col_dt)
    eT = np.ascontiguousarray((e_all.T * s).astype(ndt))  # [D, N] quantized
    wq = np.stack([(w_r * s).astype(ndt), (w_l * s).astype(ndt)], axis=1)

    in_maps = []
    for core in range(NCORES):
        aug = np.empty((P, RPC + 2), ndt)
        aug[:, 0:2] = wq
        aug[:, 2:] = eT[:, core * RPC : (core + 1) * RPC]
        in_maps.append({"et": np.ascontiguousarray(aug)})

    aux = {"rw": rw, "cw": cw, "b": b, "inv_s2": np.float32(1.0 / (s * s))}
    return in_maps, aux


def host_finish(core_outs, aux):
    """Gather/unshard: 8-scalar max across shards + elementwise finish.

    core_outs[c] is core c's [128, 64] f32 block: [p, c] (c<8) = a-dot of
    shard entity c*128+p (x S^2), [p, 8] = shard-local col max (x S^2);
    columns 9..63 are descriptor-padding and ignored.
    """
    inv_s2 = aux["inv_s2"]
    colmax = max(o[:, OWN].max() for o in core_outs) * inv_s2
    a = np.concatenate(
        [o[:, 0:OWN].T.reshape(-1) for o in core_outs]
    ) * inv_s2

    def sigmoid(x):
        return 1.0 / (1.0 + np.exp(-x))

    k1 = a + (aux["rw"] + aux["b"] + colmax)
    k2 = sigmoid(np.float32(aux["cw"] + aux["b"]) + colmax)
    return (sigmoid(k1) * k2).astype(np.float32)


def run(inputs, trace=False, trace_kwargs=None, repeat=1, col_dt=COL_DT):
    from concourse.bass_utils import run_bass_kernel_spmd

    nc = get_nc(repeat, col_dt)
    in_maps, aux = prep(**inputs, col_dt=col_dt)
    res = run_bass_kernel_spmd(
        nc,
        in_maps,
        core_ids=list(range(NCORES)),
        trace=trace,
        **(trace_kwargs or {}),
    )
    out = host_finish(
        [np.asarray(res.results[c]["out"], np.float32) for c in range(NCORES)],
        aux,
    )
    return out, res


def kernel(**inputs) -> np.ndarray:
    out, _ = run(inputs, trace=False)
    return out


# revision 34
# speedup vs baseline: 1.2870x; 1.0449x over previous
"""FALCON ObjectSomeValuesFrom forward kernel for Trainium2 (Bass, raw).

Math: the reference computes
    c_fs[j]   = sigmoid(cw + col_j + b)
    r_fs[i,j] = sigmoid(row_i + col_j + b)
    out[i]    = max_j r_fs[i,j] * c_fs[j]
with col_j = e_j . w_r, row_i = e_i . w_l + rw, cw = c_emb . w_l,
rw = r_emb . w_l.  Both product factors are strictly increasing in col_j,
so the max over j is attained at argmax_j col_j for every i:
    out[i] = sigmoid(a_i + rw + colmax + b) * sigmoid(cw + colmax + b)
with a_i = e_i . w_l and colmax = max_j col_j.  The O(N^2) pairwise block
collapses to two GEMVs over e_all plus an elementwise sigmoid tail.

Sharding: the entity axis is split 8 ways (1024 rows per core).  Each
core loads ONLY its own shard — [128, 2+1024] fp8-e3m4 (w_r, w_l in the
first two columns, the transposed shard after) — computes both GEMVs as
8 N=2 PE matmuls into one interleaved PSUM tile ([p, 2c] = col-dot,
[p, 2c+1] = a-dot of shard entity c*128+p), then in parallel: the
Activation engine copies the a-dots PSUM->SBUF while DVE max-reduces
the col-dots (the shard-local max-reduction over j).  The host-side gather/unshard combines the 8 shards: colmax = max
over the 8x128 partial maxima, then the elementwise sigmoid finish over
the gathered a-vector.  No cross-device communication — the cross-core
max rides the output gather, per the sharding plan (a device collective
would cost ~28 us of AllReduce overhead, far more than it saves).

Why raw Bass (no TileContext): the kernel is pure fixed latency — one
365 ns input DMA (+650 SEQ, +650 DGE, +900 completion semaphore),
~70 ns of matmuls, ~520 ns of tail compute+sem-props, then the output
DMA chain (625 HWDGE + 650 DGE + 900 sem) — so framework overhead IS
the cost.  Hand-rolled semaphores shed the TileContext's extra
pre/mid/post barriers and sync-point instructions (~230 ns), and
suppressing the Bass constructor's const-AP preamble (4 memsets + an
all-engine barrier this kernel never needs) lets every engine start at
t~0, shifting the whole pipeline ~640 ns left.  (A prepped-SWDGE
scatter + trigger_dma output path that also skips the 1275 ns
HWDGE+DGE stages simulates at 5149 ns and is CoreSim-exact, but the
axon/NRT backend dies with NRT_EXEC_UNIT_UNRECOVERABLE executing
gen_mode=1 preps, so it cannot be shipped.)

Sem protocol (one per edge; producers signal via drain().then_inc —
an engine-empty drain's SEQ-side update propagates ~90 ns faster than
an engine-op-attached update; sems reset by an end barrier + range
clear whose cross-engine rendezvous overlaps the final DMA wait):
  SP:   dma_start(et).then_inc(in_sem, 16)
  PE:   wait in_sem; 8x N=2 matmuls; drain (+pe_sem)
  Act:  wait pe_sem; copy a-dots -> ot[:, 0:8]; drain (+act_sem)
  DVE:  wait pe_sem; reduce_max col-dots -> ot[:, 8]; drain (+dve_sem)
  SP:   wait act+dve; dma_start(out); wait dma_sem; barrier; sem clear
"""

import numpy as np

N = 8192        # 8000 named + 192 anon entities
D = 128         # emb dim == contraction == partitions
P = 128
NCORES = 8
RPC = N // NCORES     # rows per core (1024)
OWN = RPC // P        # 8 chunks of 128 rows per core
ELEM = 16             # out columns (64B/partition keeps min-time descriptors)
COL_DT = "fp8e3"      # "fp8e4" | "fp8e3" | "fp16" | "bf16"
SCALE = {"fp8e4": 8.0, "fp8e3": 32.0, "fp16": 1.0, "bf16": 1.0}

_CACHE = {}


def _np_dt(col_dt):
    import ml_dtypes
    return {
        "fp8e4": ml_dtypes.float8_e4m3,
        "fp8e3": ml_dtypes.float8_e3m4,
        "fp16": np.float16,
        "bf16": ml_dtypes.bfloat16,
    }[col_dt]


def _build_nc(repeat=1, col_dt=COL_DT):
    import concourse.bass as bass  # noqa: F401  (env preload)
    import concourse.bacc as bacc
    import concourse.mybir as mybir

    f32 = mybir.dt.float32
    cdt = {
        "fp8e4": mybir.dt.float8e4,
        "fp8e3": mybir.dt.float8e3,
        "fp16": mybir.dt.float16,
        "bf16": mybir.dt.bfloat16,
    }[col_dt]

    # The Bass constructor unconditionally emits 4 const-AP memsets plus an
    # all-engine barrier — ~640 ns of preamble before any engine may start.
    # This kernel never reads the const APs (the one zero-source it needs is
    # hand-rolled below with a single-sem handshake), so suppress that
    # emission during construction; every engine then starts at t~0 and the
    # input DMA issues ~640 ns earlier.
    _orig_memset = bass.BassGpSimd.memset
    _orig_barrier = bass.Bass.all_engine_barrier
    bass.BassGpSimd.memset = lambda self, ap, c: None
    bass.Bass.all_engine_barrier = lambda self: None
    try:
        nc = bacc.Bacc("TRN2", target_bir_lowering=False, debug=False)
    finally:
        bass.BassGpSimd.memset = _orig_memset
        bass.Bass.all_engine_barrier = _orig_barrier

    # Cols 0:2 = [w_r, w_l] (scaled); 2:RPC+2 = this core's shard of the
    # scaled/transposed e-table.  One DMA covers every input.
    et_d = nc.dram_tensor("et", [P, RPC + 2], cdt, kind="ExternalInput").ap()
    out_d = nc.dram_tensor("out", [P, ELEM], f32, kind="ExternalOutput").ap()

    et = nc.alloc_sbuf_tensor("etsb", [P, RPC + 2], cdt).ap()
    ot = nc.alloc_sbuf_tensor("ot", [P, ELEM], f32).ap()
    zsrc = nc.alloc_sbuf_tensor("zsrc", [P, 1], f32).ap()
    dum2 = nc.alloc_sbuf_tensor("dum2", [P, 1], f32).ap()
    pst = nc.alloc_psum_tensor("pst", [P, 2 * OWN], f32).ap()
    psv = pst.rearrange("p (n two) -> p n two", two=2)

    z_sem = nc.alloc_semaphore("z_sem")
    in_sem = nc.alloc_semaphore("in_sem")
    pe_sem = nc.alloc_semaphore("pe_sem")
    act_sem = nc.alloc_semaphore("act_sem")
    dve_sem = nc.alloc_semaphore("dve_sem")
    dma_sem = nc.alloc_semaphore("dma_sem")
    sems = [z_sem, in_sem, pe_sem, act_sem, dve_sem, dma_sem]

    # Start-of-run sem resets, self-served by each sem's waiter(s) as that
    # engine's first instructions (in-order before the wait, so a stale
    # value from a previous execution can never be consumed; every updater
    # fires >2.5us into a run, every clear <100ns in).  Replaces the 233ns
    # end-of-program barrier + range clear.
    nc.sync.dma_start(et, et_d).then_inc(in_sem, 16)  # in-DMA first: t=0
    nc.sync.sem_clear(act_sem)
    nc.sync.sem_clear(dve_sem)
    nc.sync.sem_clear(dma_sem)

    # DVE: zero the pad columns of the staging tile (cols 9..15) and the
    # dummy-copy source
    nc.vector.sem_clear(pe_sem)
    nc.vector.memset(ot[:, OWN + 1 :], 0.0)
    nc.vector.memset(zsrc, 0.0).then_inc(z_sem, 1)

    # Act: dependency-free dummy copy hoists the act-table load (which the
    # compiler inserts before the first Activation) into the DMA window
    nc.scalar.sem_clear(z_sem)
    nc.scalar.sem_clear(pe_sem)
    nc.scalar.wait_ge(z_sem, 1)
    nc.scalar.copy(dum2, zsrc)

    # PE: both GEMVs as 8 N=2 matmuls ([p, 2c]=col-dot, [p, 2c+1]=a-dot).
    # One PSUM tile: raw bass has no cross-engine PSUM read hazard (the
    # Tile framework's false serialization forced a two-tile split).
    nc.tensor.sem_clear(in_sem)
    nc.tensor.wait_ge(in_sem, 16)
    w2 = et[:, 0:2]
    for _ in range(repeat):
        for c in range(OWN):
            chunk = et[:, 2 + c * P : 2 + (c + 1) * P]
            nc.tensor.matmul(psv[:, c, :], chunk, w2,
                             start=True, stop=True)
    # a single PE drain signals engine-empty (all matmuls retired);
    # its SEQ-side update propagates faster than an engine-op update
    nc.tensor.drain().then_inc(pe_sem, 1)

    # Act: a-dots copy PSUM->SBUF (drain-signaled)
    nc.scalar.wait_ge(pe_sem, 1)
    nc.scalar.copy(ot[:, 0:OWN], psv[:, :, 1])
    nc.scalar.drain().then_inc(act_sem, 1)

    # DVE: shard-local max over j (drain-signaled)
    nc.vector.wait_ge(pe_sem, 1)
    nc.vector.reduce_max(ot[:, OWN : OWN + 1], psv[:, :, 0],
                         axis=mybir.AxisListType.X)
    nc.vector.drain().then_inc(dve_sem, 1)

    # SP: the output DMA, gated on both producers
    nc.sync.wait_ge(act_sem, 1)
    nc.sync.wait_ge(dve_sem, 1)
    nc.sync.dma_start(out_d, ot).then_inc(dma_sem, 16)
    nc.sync.wait_ge(dma_sem, 16)   # output completeness; program ends here

    nc.compile()
    return nc


def get_nc(repeat=1, col_dt=COL_DT):
    key = ("nc", repeat, col_dt)
    if key not in _CACHE:
        _CACHE[key] = _build_nc(repeat, col_dt)
    return _CACHE[key]


def prep(anon_e_emb, e_table, c_table, r_table, fc0_w, fc0_b, c_id, r_id,
         col_dt=COL_DT):
    """Host prep: shard + quantize inputs; return (in_maps, aux for finish)."""
    e_all = np.concatenate(
        [np.asarray(e_table, np.float32), np.asarray(anon_e_emb, np.float32)], 0
    )  # [N, D]
    fc0_w = np.asarray(fc0_w, np.float32)
    w_l = fc0_w[0, :D]
    w_r = fc0_w[0, D:]
    b = np.float32(np.asarray(fc0_b, np.float32)[0])
    c_emb = np.asarray(c_table, np.float32)[int(c_id)]
    r_emb = np.asarray(r_table, np.float32)[int(r_id)]
    rw = np.float32(np.dot(r_emb, w_l))
    cw = np.float32(np.dot(c_emb, w_l))

    s = SCALE[col_dt]
    ndt = _np_dt(col_dt)
    eT = np.ascontiguousarray((e_all.T * s).astype(ndt))  # [D, N] quantized
    wq = np.stack([(w_r * s).astype(ndt), (w_l * s).astype(ndt)], axis=1)

    in_maps = []
    for core in range(NCORES):
        aug = np.empty((P, RPC + 2), ndt)
        aug[:, 0:2] = wq
        aug[:, 2:] = eT[:, core * RPC : (core + 1) * RPC]
        in_maps.append({"et": np.ascontiguousarray(aug)})

    aux = {"rw": rw, "cw": cw, "b": b, "inv_s2": np.float32(1.0 / (s * s))}
    return in_maps, aux


def host_finish(core_outs, aux):
    """Gather/unshard: 8-scalar max across shards + elementwise finish.

    core_outs[c] is core c's [128, 64] f32 block: [p, c] (c<8) = a-dot of
    shard entity c*128+p (x S^2), [p, 8] = shard-local col max (x S^2);
    columns 9..63 are descriptor-padding and ignored.
    """
    inv_s2 = aux["inv_s2"]
    colmax = max(o[:, OWN].max() for o in core_outs) * inv_s2
    a = np.concatenate(
        [o[:, 0:OWN].T.reshape(-1) for o in core_outs]
    ) * inv_s2

    def sigmoid(x):
        return 1.0 / (1.0 + np.exp(-x))

    k1 = a + (aux["rw"] + aux["b"] + colmax)
    k2 = sigmoid(np.float32(aux["cw"] + aux["b"]) + colmax)
    return (sigmoid(k1) * k2).astype(np.float32)


def run(inputs, trace=False, trace_kwargs=None, repeat=1, col_dt=COL_DT):
    from concourse.bass_utils import run_bass_kernel_spmd

    nc = get_nc(repeat, col_dt)
    in_maps, aux = prep(**inputs, col_dt=col_dt)
    res = run_bass_kernel_spmd(
        nc,
        in_maps,
        core_ids=list(range(NCORES)),
        trace=trace,
        **(trace_kwargs or {}),
    )
    out = host_finish(
        [np.asarray(res.results[c]["out"], np.float32) for c in range(NCORES)],
        aux,
    )
    return out, res


def kernel(**inputs) -> np.ndarray:
    out, _ = run(inputs, trace=False)
    return out
